# revision 20
# baseline (speedup 1.0000x reference)
"""Trainium2 Bass kernel for nn_DynaResidualBlockX (hypernet + per-sample 1x1 conv residual block).

Strategy (8 NeuronCores):
  - Hypernet `ks = lat @ W.T + b` is sharded by W *rows*: each core computes
    1/8 of the per-sample conv kernels for ALL 16 samples (reads 1/8 of W).
  - W rows are permuted + padded on the host ("W2" layout) so the hypernet
    matmul directly produces each conv-weight matrix in the transposed [K, M]
    layout the tensor engine wants, 128-row-aligned per output column.
  - Three AllToAll exchanges (G1 = k_in, G2 = k_mida, G3 = k_midb/k_out/
    k_short) hand core c the kernel set for its 2 samples; conv layers start
    as soon as their group's exchange lands, overlapping later exchanges.
  - Conv phase: per-sample 1x1 convs (= matmuls over the 16384 pixels),
    relu+bias fused, alternating between DVE and ACT engines per psum tile.
"""

import os
import sys

if "/opt/trn_rl_repo" not in sys.path:
    sys.path.insert(0, "/opt/trn_rl_repo")

import numpy as np

# ---------------- problem constants (hardcoded per contract) ----------------
B, FIN, FOUT, FH, LAT = 16, 64, 64, 128, 512
HWP = 128 * 128  # pixels per image
NCORE, BC = 8, 2  # cores, samples per core
# exchange groups: G1 = kin(+b_in row), G2 = mida + bmida, G3 = midb/kout/ksh
GT = [136, 136, 264]  # tiles per group (8-divisible)
GPC = [g // NCORE for g in GT]  # tiles per core per group: 17, 17, 33
TILES = sum(GT)  # 536
KT2 = TILES * 128
GOFF = [0, GT[0], GT[0] + GT[1]]  # group tile offsets
JP = 2048  # conv pixel chunk
NJ = HWP // JP  # 8 chunks
NP = 1024  # psum tile / act-op granularity
MMN = 512  # matmul moving free-dim (PSUM bank limit)
# W-chunk sizes per group (per core)
GCHUNKS = [[17], [17], [17, 16]]

# dtype mode: "bf16" (fast, ~3e-3 abs-rel err) or "f32r" (~2e-4 err)
DT_MODE = os.environ.get("KERNEL_DT", "f32r")

# tile bases within groups
G2_BMIDA = 128
G3_MIDB, G3_KOUT, G3_KSH, G3_BMIDB = 0, 128, 192, 256

S128 = 1.0 / np.sqrt(128.0)
S64 = 1.0 / 8.0


def _np_dt():
    if DT_MODE == "bf16":
        import ml_dtypes

        return ml_dtypes.bfloat16
    return np.float32


def _build_w2b2(W, b):
    """Permute/pad/scale hypernet weights into the device tile layout.

    Row r = t*128 + p of W2 produces ks-tile t, partition p; tile groups:
    [0,136) G1 kin, [136,272) G2 mida, [272,536) G3 rest.
    Returns W2 [KT2, LAT] and b2 [KT2].
    """
    r = np.arange(KT2)
    t, p = r >> 7, r & 127
    src = np.full(KT2, -1, np.int64)
    scale = np.ones(KT2, np.float32)

    # G1: kin
    m = (t < 128) & (p < 64)
    src[m] = t[m] * 64 + p[m]
    scale[m] = S128
    m = (t < 128) & (p == 64)  # b_in on the ones-channel row
    src[m] = 53248 + t[m]
    # G2: mida + bmida
    u = t - GOFF[1]
    m = (u >= 0) & (u < 128)
    src[m] = 8192 + u[m] * 128 + p[m]
    scale[m] = S128
    m = u == G2_BMIDA
    src[m] = 53376 + p[m]
    # G3: midb, kout, kshort(+bos), bmidb
    w = t - GOFF[2]
    m = (w >= 0) & (w < 128)
    src[m] = 24576 + w[m] * 128 + p[m]
    scale[m] = S128
    m = (w >= 128) & (w < 192)
    src[m] = 40960 + (w[m] - 128) * 128 + p[m]
    scale[m] = S64
    m = (w >= 192) & (w < 256) & (p < 64)
    src[m] = 49152 + (w[m] - 192) * 64 + p[m]
    scale[m] = S64
    m_bos = (w >= 192) & (w < 256) & (p == 64)
    src[m_bos] = 53632 + (w[m_bos] - 192)
    m = w == G3_BMIDB
    src[m] = 53504 + p[m]

    W2 = np.zeros((KT2, LAT), np.float32)
    b2 = np.zeros(KT2, np.float32)
    v = src >= 0
    W2[v] = W[src[v]] * scale[v][:, None]
    b2[v] = b[src[v]] * scale[v]
    W2[m_bos] += W[53696 + (w[m_bos] - 192)]
    b2[m_bos] += b[53696 + (w[m_bos] - 192)]
    return W2, b2


def _host_inputs(x, lat, W, b):
    """Build the 8 per-core input maps (pure layout work, no math)."""
    ndt = _np_dt()
    x = np.ascontiguousarray(x, np.float32).reshape(B, FIN, HWP)
    lat = np.ascontiguousarray(lat, np.float32)
    W2, b2 = _build_w2b2(np.asarray(W, np.float32), np.asarray(b, np.float32))

    latt = np.ascontiguousarray(
        lat.T.reshape(4, 128, 16).transpose(1, 0, 2).reshape(128, 64).astype(ndt)
    )

    def dup(bvec, ng):  # [ng*128] -> [128, ng*2]
        return np.ascontiguousarray(
            np.repeat(bvec.reshape(ng, 128).T[:, :, None], 2, axis=2)
            .reshape(128, ng * 2)
            .astype(ndt)
        )

    b2d = {}
    for g in range(3):
        lo = GOFF[g] * 128
        b2d[g] = dup(b2[lo : lo + GT[g] * 128], GT[g])

    xs_all = np.concatenate([x, np.ones((B, 1, HWP), np.float32)], axis=1).astype(ndt)

    def wshard(g, c):  # -> [128, GPC[g]*512] in [q, t, l, m] order
        ksg = GPC[g] * 128
        lo = GOFF[g] * 128 + c * ksg
        sh = W2[lo : lo + ksg]
        return (
            sh.reshape(GPC[g], 128, 4, 128)
            .transpose(3, 0, 2, 1)
            .reshape(128, GPC[g] * 512)
        )

    in_maps = []
    for c in range(NCORE):
        wmain = np.ascontiguousarray(
            np.concatenate([wshard(g, c) for g in range(3)], axis=1).astype(ndt)
        )
        in_maps.append(
            {
                "wmain": wmain,
                "latt": latt,
                "b2dup1": b2d[0],
                "b2dup2": b2d[1],
                "b2dup3": b2d[2],
                "xs": np.ascontiguousarray(xs_all[c * BC : (c + 1) * BC]),
            }
        )
    return in_maps


def emulate(x, lat, W, b):
    """Numpy emulation of the exact device dataflow (for layout validation)."""
    x = np.asarray(x, np.float32).reshape(B, FIN, HWP)
    W2, b2 = _build_w2b2(np.asarray(W, np.float32), np.asarray(b, np.float32))
    ksT = W2 @ np.asarray(lat, np.float32).T + b2[:, None]  # [KT2, 16]
    v1 = ksT[: GT[0] * 128].reshape(GT[0], 128, B)
    v2 = ksT[GT[0] * 128 : GOFF[2] * 128].reshape(GT[1], 128, B)
    v3 = ksT[GOFF[2] * 128 :].reshape(GT[2], 128, B)
    out = np.zeros((B, FOUT, HWP), np.float32)
    for bi in range(B):
        xb = np.concatenate([x[bi], np.ones((1, HWP), np.float32)], axis=0)
        h1 = np.maximum(v1[0:128, 0:65, bi].T.T @ xb, 0.0)
        bma = v2[G2_BMIDA, :, bi][:, None]
        h2 = np.maximum(v2[0:128, :, bi].T.T @ h1 + bma, 0.0)
        bmb = v3[G3_BMIDB, :, bi][:, None]
        h3 = np.maximum(v3[G3_MIDB : G3_MIDB + 128, :, bi].T.T @ h2 + bmb, 0.0)
        a_out = v3[G3_KOUT : G3_KOUT + 64, :, bi]
        a_sh = v3[G3_KSH : G3_KSH + 64, 0:65, bi]
        out[bi] = a_out @ h3 + a_sh @ xb
    return out.reshape(B, FOUT, 128, 128)


# ---------------------------- bass program ----------------------------------

def _build_nc():
    import concourse.bass as bass
    import concourse.tile as tile
    from concourse import bacc, mybir

    F32 = mybir.dt.float32
    DT = mybir.dt.bfloat16 if DT_MODE == "bf16" else mybir.dt.float32r
    AF = mybir.ActivationFunctionType
    ALU = mybir.AluOpType
    HB = 8 if DT_MODE == "bf16" else 5  # h-tile bufs
    XB = 4 if DT_MODE == "bf16" else 3

    nc = bacc.Bacc("TRN2", target_bir_lowering=False, debug=False, num_devices=NCORE)

    wtot = sum(GPC) * 512  # per-partition W elems
    wmain = nc.dram_tensor("wmain", [128, wtot], DT, kind="ExternalInput")
    latt = nc.dram_tensor("latt", [128, 64], DT, kind="ExternalInput")
    b2d = [
        nc.dram_tensor(f"b2dup{g + 1}", [128, GT[g] * 2], DT, kind="ExternalInput")
        for g in range(3)
    ]
    xs = nc.dram_tensor("xs", [BC, FIN + 1, HWP], DT, kind="ExternalInput")
    outd = nc.dram_tensor("out", [BC * FOUT, HWP], F32, kind="ExternalOutput")

    with tile.TileContext(nc) as tc:
        with (
            tc.tile_pool(name="persist", bufs=1) as pp,
            tc.tile_pool(name="wpool", bufs=2) as wp,
            tc.tile_pool(name="conv", bufs=2) as cp,
            tc.tile_pool(name="ps", bufs=4, space="PSUM") as psp,
            tc.tile_pool(name="dram", bufs=1, space="DRAM") as dp,
        ):
            latt_sb = pp.tile([128, 64], DT)
            nc.scalar.dma_start(latt_sb[:], latt[:])

            wm_off = [0]
            for g in range(3):
                wm_off.append(wm_off[-1] + GPC[g] * 512)

            def hyper_group(g, ks_sh_v):
                toff = 0
                for ci, csz in enumerate(GCHUNKS[g]):
                    wt = wp.tile([128, 17 * 512], DT, tag="wt", name=f"wt{g}{ci}")
                    lo = wm_off[g] + toff * 512
                    nc.sync.dma_start(
                        wt[:, 0 : csz * 512], wmain[:, lo : lo + csz * 512]
                    )
                    wtv = wt.rearrange("p (t l m) -> p t l m", t=17, l=4)
                    pk = psp.tile([128, csz * 16], F32, tag="ps", name=f"pk{g}{ci}")
                    for tl in range(csz):
                        for l in range(4):
                            nc.tensor.matmul(
                                pk[:, tl * 16 : (tl + 1) * 16],
                                wtv[:, tl, l, :],
                                latt_sb[:, l * 16 : (l + 1) * 16],
                                start=(l == 0),
                                stop=(l == 3),
                            )
                    pkv = pk.rearrange("p (t d s) -> p d t s", t=csz, d=NCORE, s=BC)
                    nc.vector.tensor_copy(
                        ks_sh_v[:, :, toff : toff + csz, :], pkv
                    )
                    toff += csz

            def exchange(g, ks_sh_v, name):
                tpc = GPC[g]
                cc_in = dp.tile([NCORE, 128, tpc * BC], DT, name=f"cc_in{name}")
                nc.sync.dma_start(
                    cc_in[:].rearrange("d p r -> p d r"),
                    ks_sh_v.rearrange("p d t s -> p d (t s)"),
                )
                cc_out = dp.tile([NCORE, 128, tpc * BC], DT, name=f"cc_out{name}")
                nc.gpsimd.collective_compute(
                    "AllToAll",
                    ALU.bypass,
                    replica_groups=[list(range(NCORE))],
                    ins=[cc_in.opt()],
                    outs=[cc_out.opt()],
                )
                ksraw = pp.tile([128, GT[g] * BC], DT, name=f"ksraw{name}")
                nc.sync.dma_start(
                    ksraw.rearrange("p (c r) -> p c r", c=NCORE),
                    cc_out[:].rearrange("c p r -> p c r"),
                )
                b2sb = pp.tile([128, GT[g] * 2], DT, name=f"b2sb{name}")
                nc.sync.dma_start(b2sb[:], b2d[g][:])
                ks = pp.tile([128, GT[g] * BC], DT, name=f"ks{name}")
                nc.vector.tensor_tensor(ks[:], ksraw[:], b2sb[:], op=ALU.add)
                return ks

            kss = []
            for g in range(3):
                ks_sh = pp.tile([128, GPC[g] * 16], DT, name=f"ks_sh{g}")
                ks_sh_v = ks_sh.rearrange(
                    "p (d t s) -> p d t s", d=NCORE, t=GPC[g], s=BC
                )
                hyper_group(g, ks_sh_v)
                kss.append(exchange(g, ks_sh_v, str(g)))

            kv1 = kss[0].rearrange("p (t s) -> p t s", t=GT[0], s=BC)
            kv2 = kss[1].rearrange("p (t s) -> p t s", t=GT[1], s=BC)
            kv3 = kss[2].rearrange("p (t s) -> p t s", t=GT[2], s=BC)
            bias4 = pp.tile([128, 4], F32)
            nc.vector.tensor_copy(bias4[:, 0:2], kss[1][:, G2_BMIDA * 2 : G2_BMIDA * 2 + 2])
            nc.vector.tensor_copy(bias4[:, 2:4], kss[2][:, G3_BMIDB * 2 : G3_BMIDB * 2 + 2])

            # ---- conv: 2 samples x 8 pixel chunks
            ek = 0  # DVE/ACT alternation counter

            def relu_bias(dst, src, bias):
                nonlocal ek
                ek += 1
                if ek % 2 == 0:
                    if bias is None:
                        nc.vector.tensor_scalar_max(dst, src, 0.0)
                    else:
                        nc.vector.tensor_scalar(
                            dst, src, bias, 0.0, op0=ALU.add, op1=ALU.max
                        )
                else:
                    if bias is None:
                        nc.scalar.activation(dst, src, AF.Relu)
                    else:
                        nc.scalar.activation(dst, src, AF.Relu, bias=bias)

            def copy_out(dst, src):
                nonlocal ek
                ek += 1
                if ek % 2 == 0:
                    nc.vector.tensor_copy(dst, src)
                else:
                    nc.scalar.activation(dst, src, AF.Copy)

            def layer(dst_h, lhsT, rhs_tile, bias, kparts):
                for half in range(JP // NP):
                    ph = psp.tile([128, NP], F32, tag="ps", name=f"ph{half}")
                    for n in range(NP // MMN):
                        lo = half * NP + n * MMN
                        nc.tensor.matmul(
                            ph[:, n * MMN : (n + 1) * MMN],
                            lhsT,
                            rhs_tile[0:kparts, lo : lo + MMN],
                            start=True,
                            stop=True,
                        )
                    hsl = slice(half * NP, (half + 1) * NP)
                    relu_bias(dst_h[:, hsl], ph[:, 0:NP], bias)

            for j in range(NJ):
                jsl = slice(j * JP, (j + 1) * JP)
                xcs, h1s, h2s, h3s = [], [], [], []
                for s in range(BC):
                    xc = cp.tile([FIN + 1, JP], DT, tag="xc", bufs=XB, name=f"xc{s}")
                    nc.sync.dma_start(xc[:], xs[s, :, jsl])
                    xcs.append(xc)
                for s in range(BC):
                    h1 = cp.tile([128, JP], DT, tag="h", bufs=HB, name=f"h1_{s}")
                    layer(h1, kv1[0:65, 0:128, s], xcs[s], None, 65)
                    h1s.append(h1)
                for s in range(BC):
                    h2 = cp.tile([128, JP], DT, tag="h", bufs=HB, name=f"h2_{s}")
                    layer(h2, kv2[:, 0:128, s], h1s[s], bias4[:, s : s + 1], 128)
                    h2s.append(h2)
                for s in range(BC):
                    h3 = cp.tile([128, JP], DT, tag="h", bufs=HB, name=f"h3_{s}")
                    layer(h3, kv3[:, G3_MIDB : G3_MIDB + 128, s], h2s[s], bias4[:, 2 + s : 3 + s], 128)
                    h3s.append(h3)
                # out layer: k_out @ h3 + k_short' @ [x; 1]  (bias via ones row)
                if DT_MODE == "bf16":
                    # both samples into one [128, JP] psum via column tiling
                    oc = cp.tile([128, JP], F32, tag="oc", bufs=2, name="oc")
                    for half in range(JP // NP):
                        po = psp.tile([128, NP], F32, tag="ps", name=f"po{half}")
                        for s in range(BC):
                            for n in range(NP // MMN):
                                lo = half * NP + n * MMN
                                nsl = slice(n * MMN, (n + 1) * MMN)
                                nc.tensor.matmul(
                                    po[s * 64 : (s + 1) * 64, nsl],
                                    kv3[:, G3_KOUT : G3_KOUT + 64, s],
                                    h3s[s][:, lo : lo + MMN],
                                    start=True,
                                    stop=False,
                                    tile_position=(0, s * 64),
                                )
                                nc.tensor.matmul(
                                    po[s * 64 : (s + 1) * 64, nsl],
                                    kv3[0:65, G3_KSH : G3_KSH + 64, s],
                                    xcs[s][:, lo : lo + MMN],
                                    start=False,
                                    stop=True,
                                    tile_position=(0, s * 64),
                                )
                        hsl = slice(half * NP, (half + 1) * NP)
                        copy_out(oc[:, hsl], po[:, 0:NP])
                    nc.sync.dma_start(outd[:, jsl], oc[:])
                else:
                    for s in range(BC):
                        oc = cp.tile([64, JP], F32, tag="oc", bufs=4, name=f"oc{s}")
                        for half in range(JP // NP):
                            po = psp.tile([64, NP], F32, tag="ps", name=f"po{half}")
                            for n in range(NP // MMN):
                                lo = half * NP + n * MMN
                                nsl = slice(n * MMN, (n + 1) * MMN)
                                nc.tensor.matmul(
                                    po[:, nsl],
                                    kv3[:, G3_KOUT : G3_KOUT + 64, s],
                                    h3s[s][:, lo : lo + MMN],
                                    start=True,
                                    stop=False,
                                )
                                nc.tensor.matmul(
                                    po[:, nsl],
                                    kv3[0:65, G3_KSH : G3_KSH + 64, s],
                                    xcs[s][:, lo : lo + MMN],
                                    start=False,
                                    stop=True,
                                )
                            hsl = slice(half * NP, (half + 1) * NP)
                            copy_out(oc[:, hsl], po[:, 0:NP])
                        nc.sync.dma_start(outd[s * 64 : (s + 1) * 64, jsl], oc[:])

    nc.compile()
    return nc


_NC_CACHE = None


def kernel(x, lat, W, b):
    from concourse.bass_utils import run_bass_kernel_spmd

    global _NC_CACHE
    if _NC_CACHE is None:
        _NC_CACHE = _build_nc()
    nc = _NC_CACHE
    in_maps = _host_inputs(x, lat, W, b)
    res = run_bass_kernel_spmd(nc, in_maps, core_ids=list(range(NCORE)))
    out = np.concatenate([res.results[c]["out"] for c in range(NCORE)], axis=0)
    return np.ascontiguousarray(out.reshape(B, FOUT, 128, 128))


# revision 24
# speedup vs baseline: 1.0700x; 1.0700x over previous
"""Trainium2 Bass kernel for nn_DynaResidualBlockX (hypernet + per-sample 1x1 conv residual block).

Strategy (8 NeuronCores):
  - Hypernet `ks = lat @ W.T + b` is sharded by W *rows*: each core computes
    1/8 of the per-sample conv kernels for ALL 16 samples (reads 1/8 of W).
  - W rows are permuted + padded on the host ("W2" layout) so the hypernet
    matmul directly produces each conv-weight matrix in the transposed [K, M]
    layout the tensor engine wants, 128-row-aligned per output column.
  - Three AllToAll exchanges (G1 = k_in, G2 = k_mida, G3 = k_midb/k_out/
    k_short) hand core c the kernel set for its 2 samples; conv layers start
    as soon as their group's exchange lands, overlapping later exchanges.
  - Conv phase: per-sample 1x1 convs (= matmuls over the 16384 pixels),
    relu+bias fused, alternating between DVE and ACT engines per psum tile.
"""

import os
import sys

if "/opt/trn_rl_repo" not in sys.path:
    sys.path.insert(0, "/opt/trn_rl_repo")

import numpy as np

# ---------------- problem constants (hardcoded per contract) ----------------
B, FIN, FOUT, FH, LAT = 16, 64, 64, 128, 512
HWP = 128 * 128  # pixels per image
NCORE, BC = 8, 2  # cores, samples per core
# exchange groups: G1 = kin(+b_in row), G2 = mida + bmida, G3 = midb/kout/ksh
GT = [136, 136, 264]  # tiles per group (8-divisible)
GPC = [g // NCORE for g in GT]  # tiles per core per group: 17, 17, 33
TILES = sum(GT)  # 536
KT2 = TILES * 128
GOFF = [0, GT[0], GT[0] + GT[1]]  # group tile offsets
JP = 2048  # conv pixel chunk
NJ = HWP // JP  # 8 chunks
NP = 1024  # psum tile / act-op granularity
MMN = 512  # matmul moving free-dim (PSUM bank limit)
# W-chunk sizes per group (per core)
GCHUNKS = [[17], [17], [17, 16]]

# dtype mode: "bf16" (fast, ~3e-3 abs-rel err) or "f32r" (~2e-4 err)
DT_MODE = os.environ.get("KERNEL_DT", "bf16")

# tile bases within groups
G2_BMIDA = 128
G3_MIDB, G3_KOUT, G3_KSH, G3_BMIDB = 0, 128, 192, 256

S128 = 1.0 / np.sqrt(128.0)
S64 = 1.0 / 8.0


def _np_dt():
    if DT_MODE == "bf16":
        import ml_dtypes

        return ml_dtypes.bfloat16
    return np.float32


def _build_w2b2(W, b):
    """Permute/pad/scale hypernet weights into the device tile layout.

    Row r = t*128 + p of W2 produces ks-tile t, partition p; tile groups:
    [0,136) G1 kin, [136,272) G2 mida, [272,536) G3 rest.
    Returns W2 [KT2, LAT] and b2 [KT2].
    """
    r = np.arange(KT2)
    t, p = r >> 7, r & 127
    src = np.full(KT2, -1, np.int64)
    scale = np.ones(KT2, np.float32)

    # G1: kin
    m = (t < 128) & (p < 64)
    src[m] = t[m] * 64 + p[m]
    scale[m] = S128
    m = (t < 128) & (p == 64)  # b_in on the ones-channel row
    src[m] = 53248 + t[m]
    # G2: mida + bmida
    u = t - GOFF[1]
    m = (u >= 0) & (u < 128)
    src[m] = 8192 + u[m] * 128 + p[m]
    scale[m] = S128
    m = u == G2_BMIDA
    src[m] = 53376 + p[m]
    # G3: midb, kout, kshort(+bos), bmidb
    w = t - GOFF[2]
    m = (w >= 0) & (w < 128)
    src[m] = 24576 + w[m] * 128 + p[m]
    scale[m] = S128
    m = (w >= 128) & (w < 192)
    src[m] = 40960 + (w[m] - 128) * 128 + p[m]
    scale[m] = S64
    m = (w >= 192) & (w < 256) & (p < 64)
    src[m] = 49152 + (w[m] - 192) * 64 + p[m]
    scale[m] = S64
    m_bos = (w >= 192) & (w < 256) & (p == 64)
    src[m_bos] = 53632 + (w[m_bos] - 192)
    m = w == G3_BMIDB
    src[m] = 53504 + p[m]

    W2 = np.zeros((KT2, LAT), np.float32)
    b2 = np.zeros(KT2, np.float32)
    v = src >= 0
    W2[v] = W[src[v]] * scale[v][:, None]
    b2[v] = b[src[v]] * scale[v]
    W2[m_bos] += W[53696 + (w[m_bos] - 192)]
    b2[m_bos] += b[53696 + (w[m_bos] - 192)]
    return W2, b2


def _host_inputs(x, lat, W, b):
    """Build the 8 per-core input maps (pure layout work, no math)."""
    ndt = _np_dt()
    x = np.ascontiguousarray(x, np.float32).reshape(B, FIN, HWP)
    lat = np.ascontiguousarray(lat, np.float32)
    W2, b2 = _build_w2b2(np.asarray(W, np.float32), np.asarray(b, np.float32))

    latt = np.ascontiguousarray(
        lat.T.reshape(4, 128, 16).transpose(1, 0, 2).reshape(128, 64).astype(ndt)
    )

    def dup(bvec, ng):  # [ng*128] -> [128, ng*2]
        return np.ascontiguousarray(
            np.repeat(bvec.reshape(ng, 128).T[:, :, None], 2, axis=2)
            .reshape(128, ng * 2)
            .astype(ndt)
        )

    b2d = {}
    for g in range(3):
        lo = GOFF[g] * 128
        b2d[g] = dup(b2[lo : lo + GT[g] * 128], GT[g])

    xs_all = np.concatenate([x, np.ones((B, 1, HWP), np.float32)], axis=1).astype(ndt)

    def wshard(g, c):  # -> [128, GPC[g]*512] in [q, t, l, m] order
        ksg = GPC[g] * 128
        lo = GOFF[g] * 128 + c * ksg
        sh = W2[lo : lo + ksg]
        return (
            sh.reshape(GPC[g], 128, 4, 128)
            .transpose(3, 0, 2, 1)
            .reshape(128, GPC[g] * 512)
        )

    in_maps = []
    for c in range(NCORE):
        wmain = np.ascontiguousarray(
            np.concatenate([wshard(g, c) for g in range(3)], axis=1).astype(ndt)
        )
        in_maps.append(
            {
                "wmain": wmain,
                "latt": latt,
                "b2dup1": b2d[0],
                "b2dup2": b2d[1],
                "b2dup3": b2d[2],
                "xs": np.ascontiguousarray(xs_all[c * BC : (c + 1) * BC]),
            }
        )
    return in_maps


def emulate(x, lat, W, b):
    """Numpy emulation of the exact device dataflow (for layout validation)."""
    x = np.asarray(x, np.float32).reshape(B, FIN, HWP)
    W2, b2 = _build_w2b2(np.asarray(W, np.float32), np.asarray(b, np.float32))
    ksT = W2 @ np.asarray(lat, np.float32).T + b2[:, None]  # [KT2, 16]
    v1 = ksT[: GT[0] * 128].reshape(GT[0], 128, B)
    v2 = ksT[GT[0] * 128 : GOFF[2] * 128].reshape(GT[1], 128, B)
    v3 = ksT[GOFF[2] * 128 :].reshape(GT[2], 128, B)
    out = np.zeros((B, FOUT, HWP), np.float32)
    for bi in range(B):
        xb = np.concatenate([x[bi], np.ones((1, HWP), np.float32)], axis=0)
        h1 = np.maximum(v1[0:128, 0:65, bi].T.T @ xb, 0.0)
        bma = v2[G2_BMIDA, :, bi][:, None]
        h2 = np.maximum(v2[0:128, :, bi].T.T @ h1 + bma, 0.0)
        bmb = v3[G3_BMIDB, :, bi][:, None]
        h3 = np.maximum(v3[G3_MIDB : G3_MIDB + 128, :, bi].T.T @ h2 + bmb, 0.0)
        a_out = v3[G3_KOUT : G3_KOUT + 64, :, bi]
        a_sh = v3[G3_KSH : G3_KSH + 64, 0:65, bi]
        out[bi] = a_out @ h3 + a_sh @ xb
    return out.reshape(B, FOUT, 128, 128)


# ---------------------------- bass program ----------------------------------

def _build_nc():
    import concourse.bass as bass
    import concourse.tile as tile
    from concourse import bacc, mybir

    F32 = mybir.dt.float32
    DT = mybir.dt.bfloat16 if DT_MODE == "bf16" else mybir.dt.float32r
    AF = mybir.ActivationFunctionType
    ALU = mybir.AluOpType
    HB = 8 if DT_MODE == "bf16" else 5  # h-tile bufs
    XB = 4 if DT_MODE == "bf16" else 3

    nc = bacc.Bacc("TRN2", target_bir_lowering=False, debug=False, num_devices=NCORE)

    wtot = sum(GPC) * 512  # per-partition W elems
    wmain = nc.dram_tensor("wmain", [128, wtot], DT, kind="ExternalInput")
    latt = nc.dram_tensor("latt", [128, 64], DT, kind="ExternalInput")
    b2d = [
        nc.dram_tensor(f"b2dup{g + 1}", [128, GT[g] * 2], DT, kind="ExternalInput")
        for g in range(3)
    ]
    xs = nc.dram_tensor("xs", [BC, FIN + 1, HWP], DT, kind="ExternalInput")
    outd = nc.dram_tensor("out", [BC * FOUT, HWP], F32, kind="ExternalOutput")

    with tile.TileContext(nc) as tc:
        with (
            tc.tile_pool(name="persist", bufs=1) as pp,
            tc.tile_pool(name="wpool", bufs=2) as wp,
            tc.tile_pool(name="conv", bufs=2) as cp,
            tc.tile_pool(name="ps", bufs=4, space="PSUM") as psp,
            tc.tile_pool(name="dram", bufs=1, space="DRAM") as dp,
        ):
            latt_sb = pp.tile([128, 64], DT)
            nc.scalar.dma_start(latt_sb[:], latt[:])

            wm_off = [0]
            for g in range(3):
                wm_off.append(wm_off[-1] + GPC[g] * 512)

            def hyper_group(g, ks_sh_v):
                toff = 0
                for ci, csz in enumerate(GCHUNKS[g]):
                    wt = wp.tile([128, 17 * 512], DT, tag="wt", name=f"wt{g}{ci}")
                    lo = wm_off[g] + toff * 512
                    nc.sync.dma_start(
                        wt[:, 0 : csz * 512], wmain[:, lo : lo + csz * 512]
                    )
                    wtv = wt.rearrange("p (t l m) -> p t l m", t=17, l=4)
                    pk = psp.tile([128, csz * 16], F32, tag="ps", name=f"pk{g}{ci}")
                    for tl in range(csz):
                        for l in range(4):
                            nc.tensor.matmul(
                                pk[:, tl * 16 : (tl + 1) * 16],
                                wtv[:, tl, l, :],
                                latt_sb[:, l * 16 : (l + 1) * 16],
                                start=(l == 0),
                                stop=(l == 3),
                            )
                    pkv = pk.rearrange("p (t d s) -> p d t s", t=csz, d=NCORE, s=BC)
                    nc.vector.tensor_copy(
                        ks_sh_v[:, :, toff : toff + csz, :], pkv
                    )
                    toff += csz

            def exchange(g, ks_sh_v, name):
                tpc = GPC[g]
                cc_in = dp.tile([NCORE, 128, tpc * BC], DT, name=f"cc_in{name}")
                nc.sync.dma_start(
                    cc_in[:].rearrange("d p r -> p d r"),
                    ks_sh_v.rearrange("p d t s -> p d (t s)"),
                )
                cc_out = dp.tile([NCORE, 128, tpc * BC], DT, name=f"cc_out{name}")
                nc.gpsimd.collective_compute(
                    "AllToAll",
                    ALU.bypass,
                    replica_groups=[list(range(NCORE))],
                    ins=[cc_in.opt()],
                    outs=[cc_out.opt()],
                )
                ksraw = pp.tile([128, GT[g] * BC], DT, name=f"ksraw{name}")
                nc.sync.dma_start(
                    ksraw.rearrange("p (c r) -> p c r", c=NCORE),
                    cc_out[:].rearrange("c p r -> p c r"),
                )
                b2sb = pp.tile([128, GT[g] * 2], DT, name=f"b2sb{name}")
                nc.sync.dma_start(b2sb[:], b2d[g][:])
                ks = pp.tile([128, GT[g] * BC], DT, name=f"ks{name}")
                nc.vector.tensor_tensor(ks[:], ksraw[:], b2sb[:], op=ALU.add)
                return ks

            kss = []
            for g in range(3):
                ks_sh = pp.tile([128, GPC[g] * 16], DT, name=f"ks_sh{g}")
                ks_sh_v = ks_sh.rearrange(
                    "p (d t s) -> p d t s", d=NCORE, t=GPC[g], s=BC
                )
                hyper_group(g, ks_sh_v)
                kss.append(exchange(g, ks_sh_v, str(g)))

            kv1 = kss[0].rearrange("p (t s) -> p t s", t=GT[0], s=BC)
            kv2 = kss[1].rearrange("p (t s) -> p t s", t=GT[1], s=BC)
            kv3 = kss[2].rearrange("p (t s) -> p t s", t=GT[2], s=BC)
            bias4 = pp.tile([128, 4], F32)
            nc.vector.tensor_copy(bias4[:, 0:2], kss[1][:, G2_BMIDA * 2 : G2_BMIDA * 2 + 2])
            nc.vector.tensor_copy(bias4[:, 2:4], kss[2][:, G3_BMIDB * 2 : G3_BMIDB * 2 + 2])

            # ---- conv: 2 samples x 8 pixel chunks
            ek = 0  # DVE/ACT alternation counter

            def relu_bias(dst, src, bias):
                nonlocal ek
                ek += 1
                if ek % 2 == 0:
                    if bias is None:
                        nc.vector.tensor_scalar_max(dst, src, 0.0)
                    else:
                        nc.vector.tensor_scalar(
                            dst, src, bias, 0.0, op0=ALU.add, op1=ALU.max
                        )
                else:
                    if bias is None:
                        nc.scalar.activation(dst, src, AF.Relu)
                    else:
                        nc.scalar.activation(dst, src, AF.Relu, bias=bias)

            def copy_out(dst, src):
                nonlocal ek
                ek += 1
                if ek % 2 == 0:
                    nc.vector.tensor_copy(dst, src)
                else:
                    nc.scalar.activation(dst, src, AF.Copy)

            def layer(dst_h, lhsT, rhs_tile, bias, kparts):
                for half in range(JP // NP):
                    ph = psp.tile([128, NP], F32, tag="ps", name=f"ph{half}")
                    for n in range(NP // MMN):
                        lo = half * NP + n * MMN
                        nc.tensor.matmul(
                            ph[:, n * MMN : (n + 1) * MMN],
                            lhsT,
                            rhs_tile[0:kparts, lo : lo + MMN],
                            start=True,
                            stop=True,
                        )
                    hsl = slice(half * NP, (half + 1) * NP)
                    relu_bias(dst_h[:, hsl], ph[:, 0:NP], bias)

            for j in range(NJ):
                jsl = slice(j * JP, (j + 1) * JP)
                xcs, h1s, h2s, h3s = [], [], [], []
                for s in range(BC):
                    xc = cp.tile([FIN + 1, JP], DT, tag="xc", bufs=XB, name=f"xc{s}")
                    nc.sync.dma_start(xc[:], xs[s, :, jsl])
                    xcs.append(xc)
                for s in range(BC):
                    h1 = cp.tile([128, JP], DT, tag="h", bufs=HB, name=f"h1_{s}")
                    layer(h1, kv1[0:65, 0:128, s], xcs[s], None, 65)
                    h1s.append(h1)
                for s in range(BC):
                    h2 = cp.tile([128, JP], DT, tag="h", bufs=HB, name=f"h2_{s}")
                    layer(h2, kv2[:, 0:128, s], h1s[s], bias4[:, s : s + 1], 128)
                    h2s.append(h2)
                for s in range(BC):
                    h3 = cp.tile([128, JP], DT, tag="h", bufs=HB, name=f"h3_{s}")
                    layer(h3, kv3[:, G3_MIDB : G3_MIDB + 128, s], h2s[s], bias4[:, 2 + s : 3 + s], 128)
                    h3s.append(h3)
                # out layer: k_out @ h3 + k_short' @ [x; 1]  (bias via ones row)
                if DT_MODE == "bf16":
                    # both samples into one [128, JP] psum via column tiling
                    oc = cp.tile([128, JP], F32, tag="oc", bufs=2, name="oc")
                    for half in range(JP // NP):
                        po = psp.tile([128, NP], F32, tag="ps", name=f"po{half}")
                        for s in range(BC):
                            for n in range(NP // MMN):
                                lo = half * NP + n * MMN
                                nsl = slice(n * MMN, (n + 1) * MMN)
                                nc.tensor.matmul(
                                    po[s * 64 : (s + 1) * 64, nsl],
                                    kv3[:, G3_KOUT : G3_KOUT + 64, s],
                                    h3s[s][:, lo : lo + MMN],
                                    start=True,
                                    stop=False,
                                    tile_position=(0, s * 64),
                                )
                                nc.tensor.matmul(
                                    po[s * 64 : (s + 1) * 64, nsl],
                                    kv3[0:65, G3_KSH : G3_KSH + 64, s],
                                    xcs[s][:, lo : lo + MMN],
                                    start=False,
                                    stop=True,
                                    tile_position=(0, s * 64),
                                )
                        hsl = slice(half * NP, (half + 1) * NP)
                        copy_out(oc[:, hsl], po[:, 0:NP])
                    nc.sync.dma_start(outd[:, jsl], oc[:])
                else:
                    for s in range(BC):
                        oc = cp.tile([64, JP], F32, tag="oc", bufs=4, name=f"oc{s}")
                        for half in range(JP // NP):
                            po = psp.tile([64, NP], F32, tag="ps", name=f"po{half}")
                            for n in range(NP // MMN):
                                lo = half * NP + n * MMN
                                nsl = slice(n * MMN, (n + 1) * MMN)
                                nc.tensor.matmul(
                                    po[:, nsl],
                                    kv3[:, G3_KOUT : G3_KOUT + 64, s],
                                    h3s[s][:, lo : lo + MMN],
                                    start=True,
                                    stop=False,
                                )
                                nc.tensor.matmul(
                                    po[:, nsl],
                                    kv3[0:65, G3_KSH : G3_KSH + 64, s],
                                    xcs[s][:, lo : lo + MMN],
                                    start=False,
                                    stop=True,
                                )
                            hsl = slice(half * NP, (half + 1) * NP)
                            copy_out(oc[:, hsl], po[:, 0:NP])
                        nc.sync.dma_start(outd[s * 64 : (s + 1) * 64, jsl], oc[:])

    nc.compile()
    return nc


_NC_CACHE = None


def kernel(x, lat, W, b):
    from concourse.bass_utils import run_bass_kernel_spmd

    global _NC_CACHE
    if _NC_CACHE is None:
        _NC_CACHE = _build_nc()
    nc = _NC_CACHE
    in_maps = _host_inputs(x, lat, W, b)
    res = run_bass_kernel_spmd(nc, in_maps, core_ids=list(range(NCORE)))
    out = np.concatenate([res.results[c]["out"] for c in range(NCORE)], axis=0)
    return np.ascontiguousarray(out.reshape(B, FOUT, 128, 128))
